# revision 12
# baseline (speedup 1.0000x reference)
"""Grouped-Query Attention (B=1, L=4096, D=1024, 16 q-heads, 4 kv-heads, hd=64)
on 8 Trainium2 NeuronCores.

Sharding: core c owns q-heads {2c, 2c+1} and their shared kv-head c//2.
Host sums the 8 row-parallel out-proj partials and adds bo.

v3 per-core dataflow (bf16 PE, fp32 PSUM), ACT-bound software pipeline:
  - Wq packs both heads [1024,128]; Wk host-duplicated so one M=128
    projection writes K^T replicated on partition halves 0-63/64-127.
  - Q^T [128, L]: head0 partitions 0-63, head1 64-127, pre-scaled 1/8.
  - PSUM: 3-slot score ring (2 banks each: slice0=h0, slice1=h1) + 2 avps.
  - Per k-tile j: row-tiled score pair (h0 rows 0-63 / h1 rows 64-127,
    concurrent), ONE exp [128,1024] covering both heads, then 2 attn@V
    matmuls. Issue order V(j), S(j+3) keeps ACT 100% busy; out-proj of the
    previous q-block is dripped one matmul per k-tile into the PE slack.
  - Epilogue: denominator broadcast via rank-1 matmul (partition 64 -> 0-63)
    then reciprocal_approx_fast at partition 0 (custom DVE needs base 0).
"""

import os

os.environ.setdefault("MYCRO_LOCAL_CACHE", "1")

import numpy as np
import ml_dtypes

import concourse.bass as bass
import concourse.bacc as bacc
import concourse.mybir as mybir
from concourse.tile import TileContext
from concourse.bass_utils import run_bass_kernel_spmd

BF16 = mybir.dt.bfloat16
F32 = mybir.dt.float32
AF = mybir.ActivationFunctionType

D = 1024
L = 4096
NHEAD = 16
NKV = 4
HD = 64
NCORES = 8
HPC = NHEAD // NCORES  # 2 q heads per core
QB = 512               # q-block width
NQB = L // QB          # 8
KT = 128               # k-tile
NKT = L // KT          # 32
NF = D // 128          # 8 feature chunks
SCALE = 0.125          # 1/sqrt(64)
LOOKAHEAD = 3          # score staging ring depth

_CACHE = {}


def _build(has_bias):
    nc = bacc.Bacc("TRN2", target_bir_lowering=False, debug=False)

    xT = nc.declare_dram_parameter("xT", [D, L], BF16, isOutput=False)
    wq = nc.declare_dram_parameter("wq", [D, HPC * HD], BF16, isOutput=False)
    wk2 = nc.declare_dram_parameter("wk2", [D, 2 * HD], BF16, isOutput=False)
    wv = nc.declare_dram_parameter("wv", [D, HD], BF16, isOutput=False)
    wo0 = nc.declare_dram_parameter("wo0", [HD, D], BF16, isOutput=False)
    wo1 = nc.declare_dram_parameter("wo1", [HD, D], BF16, isOutput=False)
    bq = nc.declare_dram_parameter("bq", [1, HPC * HD], BF16, isOutput=False)
    bk = nc.declare_dram_parameter("bk", [1, 2 * HD], BF16, isOutput=False)
    bv = nc.declare_dram_parameter("bv", [1, HD], BF16, isOutput=False)
    out = nc.declare_dram_parameter("out", [L, D], F32, isOutput=True)

    with TileContext(nc) as tc:
        with (
            tc.tile_pool(name="sing", bufs=1) as sing,
            tc.tile_pool(name="ptp", bufs=8) as ptp,
            tc.tile_pool(name="attp", bufs=2) as attp,
            tc.tile_pool(name="nrm", bufs=3) as nrm,
            tc.tile_pool(name="obp", bufs=3) as obp,
            tc.tile_pool(name="psA", bufs=LOOKAHEAD, space="PSUM") as psA,
            tc.tile_pool(name="psB", bufs=2, space="PSUM") as psB,
        ):
            # ---- resident SBUF tensors ----
            xT_sb = sing.tile([128, NF, L], BF16)
            wq_sb = sing.tile([128, NF, HPC * HD], BF16)
            wk2_sb = sing.tile([128, NF, 2 * HD], BF16)
            wv_sb = sing.tile([128, NF, HD], BF16)
            wo0_sb = sing.tile([HD, D], BF16)
            wo1_sb = sing.tile([HD, D], BF16)
            ones_f = sing.tile([HD + 1, HD], F32)
            KT_sb = sing.tile([128, L], BF16)       # K^T dup on both halves
            QT_sb = sing.tile([128, L], BF16)       # h0 rows 0-63, h1 64-127
            V_sb = sing.tile([128, NKT, HD + 1], BF16)  # col 64 = 1.0 (denom)
            if has_bias:
                bq_sb = sing.tile([1, HPC * HD], BF16)
                bk_sb = sing.tile([1, 2 * HD], BF16)
                bv_sb = sing.tile([1, HD], BF16)
                ones_b = sing.tile([1, QB], BF16)

            for f in range(NF):
                fs = slice(128 * f, 128 * (f + 1))
                nc.sync.dma_start(out=xT_sb[:, f, 0:QB], in_=xT[fs, 0:QB])
                nc.sync.dma_start(out=wk2_sb[:, f, :], in_=wk2[fs, :])
                nc.sync.dma_start(out=wq_sb[:, f, :], in_=wq[fs, :])
                nc.sync.dma_start(out=wv_sb[:, f, :], in_=wv[fs, :])
            for n in range(1, NQB):
                ns = slice(QB * n, QB * (n + 1))
                for f in range(NF):
                    fs = slice(128 * f, 128 * (f + 1))
                    nc.sync.dma_start(out=xT_sb[:, f, ns], in_=xT[fs, ns])
                if n == 1:
                    nc.sync.dma_start(out=wo0_sb, in_=wo0[:, :])
                    nc.sync.dma_start(out=wo1_sb, in_=wo1[:, :])
            if has_bias:
                nc.sync.dma_start(out=bq_sb, in_=bq[:, :])
                nc.sync.dma_start(out=bk_sb, in_=bk[:, :])
                nc.sync.dma_start(out=bv_sb, in_=bv[:, :])
                nc.gpsimd.memset(ones_b, 1.0)
            nc.gpsimd.memset(ones_f, 1.0)
            nc.gpsimd.memset(V_sb[:, :, HD], 1.0)

            # ---- K/V projection issue helpers ----
            def issue_kproj(n):
                ns = slice(QB * n, QB * (n + 1))
                kps = psA.tile([128, QB], F32, tag="st", name="kps")
                for f in range(NF):
                    nc.tensor.matmul(kps, wk2_sb[:, f, :], xT_sb[:, f, ns],
                                     start=(f == 0),
                                     stop=(not has_bias and f == NF - 1))
                if has_bias:
                    nc.tensor.matmul(kps, bk_sb, ones_b, start=False, stop=True)
                nc.vector.tensor_copy(KT_sb[:, ns], kps)

            def issue_vproj(l):
                ls = slice(KT * l, KT * (l + 1))
                vps = psA.tile([128, HD], F32, tag="st", name="vps")
                for f in range(NF):
                    nc.tensor.matmul(vps, xT_sb[:, f, ls], wv_sb[:, f, :],
                                     start=(f == 0),
                                     stop=(not has_bias and f == NF - 1))
                if has_bias:
                    nc.tensor.matmul(vps, ones_b[:, 0:KT], bv_sb,
                                     start=False, stop=True)
                nc.vector.tensor_copy(V_sb[:, l, 0:HD], vps)

            # ---- software-pipelined attention over q-blocks ----
            def issue_qproj(q):
                qs = slice(QB * q, QB * (q + 1))
                qps = psA.tile([128, QB], F32, tag="st", name="qps")
                for f in range(NF):
                    nc.tensor.matmul(qps, wq_sb[:, f, :], xT_sb[:, f, qs],
                                     start=(f == 0),
                                     stop=(not has_bias and f == NF - 1))
                if has_bias:
                    nc.tensor.matmul(qps, bq_sb, ones_b,
                                     start=False, stop=True)
                nc.vector.tensor_scalar_mul(QT_sb[:, qs], qps, SCALE)

            pts = {}

            def issue_S(q, j):
                qs = slice(QB * q, QB * (q + 1))
                ks = slice(KT * j, KT * (j + 1))
                st = psA.tile([128, 2, QB], F32, tag="st", name="st")
                # row-tiled pair: h0 on PE rows 0-63, h1 on rows 64-127,
                # concurrent in disjoint row groups, disjoint PSUM banks
                nc.tensor.matmul(st[:, 0, :], KT_sb[0:HD, ks],
                                 QT_sb[0:HD, qs], start=True, stop=True)
                nc.tensor.matmul(st[:, 1, :], KT_sb[HD:128, ks],
                                 QT_sb[HD:128, qs], start=True, stop=True)
                pt = ptp.tile([128, 2, QB], BF16, tag="pt", name="pt")
                nc.scalar.activation(pt, st, AF.Exp)   # both heads, one instr
                pts[j] = pt

            def issue_V(avps, j):
                pt = pts.pop(j)
                for h in range(HPC):
                    nc.tensor.matmul(avps[h], V_sb[:, j, :], pt[:, h, :],
                                     start=(j == 0), stop=(j == NKT - 1))

            def issue_epilogue(q, atT, avps):
                for h in range(HPC):
                    # broadcast denominator (partition 64) to partitions
                    # 0-63 via rank-1 matmul, then fast reciprocal at base
                    # partition 0 (custom DVE ops require base partition 0)
                    d_sb = nrm.tile([HD + 1, QB], F32, tag="rd", name="d_sb")
                    nc.vector.tensor_copy(d_sb[HD:HD + 1, :],
                                          avps[h][HD:HD + 1, :])
                    dbps = psA.tile([HD, QB], F32, tag="st", name="dbps")
                    nc.tensor.matmul(dbps, ones_f[HD:HD + 1, :],
                                     d_sb[HD:HD + 1, :], start=True, stop=True)
                    rbsb = nrm.tile([HD, QB], F32, tag="rb_sb")
                    nc.vector.reciprocal_approx_fast(rbsb, dbps)
                    nc.vector.tensor_mul(atT[h], avps[h][0:HD, :], rbsb)

            def outproj_items(q, atT):
                # 16 single-matmul work items + copy/DMA, dripped into the
                # next q-block's k-tile loop
                items = []
                state = {}
                for lc in range(QB // 128):
                    lcs = slice(128 * lc, 128 * (lc + 1))

                    def alloc(lc=lc):
                        state["ops"] = psA.tile([128, 2, QB], F32, tag="st",
                                                name="ops")
                        state["osb"] = obp.tile([128, D], F32, tag="ob",
                                                name="osb")

                    for n in range(2):
                        ns = slice(QB * n, QB * (n + 1))
                        for h in range(HPC):
                            def mm(lc=lc, n=n, h=h, lcs=lcs, ns=ns):
                                if n == 0 and h == 0:
                                    alloc(lc)
                                wo_sb = wo0_sb if h == 0 else wo1_sb
                                nc.tensor.matmul(
                                    state["ops"][:, n, :], atT[h][:, lcs],
                                    wo_sb[:, ns], start=(h == 0),
                                    stop=(h == HPC - 1))
                                if n == 1 and h == HPC - 1:
                                    nc.vector.tensor_copy(state["osb"],
                                                          state["ops"])
                                    r0 = QB * q + 128 * lc
                                    nc.sync.dma_start(
                                        out=out[r0:r0 + 128, :],
                                        in_=state["osb"])
                            items.append(mm)
                return items

            # Flat (q, j) stream: the score/exp pipeline never drains across
            # q-block boundaries. Lookahead S issues wrap into the next
            # q-block; qproj for q+1 is issued a few k-tiles early.
            deferred = []
            issue_kproj(0)
            issue_qproj(0)
            for j in range(LOOKAHEAD):
                issue_S(0, j)
            for l in range(3):
                issue_vproj(l)
            issue_kproj(1)
            for q in range(NQB):
                atT = [attp.tile([HD, QB], BF16, tag=f"a{h}", name=f"atT{h}")
                       for h in range(HPC)]
                avps = [psB.tile([HD + 1, QB], F32, tag="av", name=f"avps{h}")
                        for h in range(HPC)]
                for j in range(NKT):
                    issue_V(avps, j)
                    if q == 0 and j + 3 < NKT:
                        issue_vproj(j + 3)
                    if q == 0 and j >= 2 and (j - 2) % 4 == 0 \
                            and (j - 2) // 4 + 2 < NQB:
                        issue_kproj((j - 2) // 4 + 2)
                    nj = j + LOOKAHEAD
                    if j == NKT - 12 and q + 1 < NQB:
                        issue_qproj(q + 1)
                    if nj < NKT:
                        issue_S(q, nj)
                    elif q + 1 < NQB:
                        issue_S(q + 1, nj - NKT)
                    if deferred and 2 <= j < 2 + len(deferred):
                        deferred[j - 2]()
                issue_epilogue(q, atT, avps)
                deferred = outproj_items(q, atT)
            for it in deferred:
                it()
    nc.finalize()
    return nc


def _prep_inputs(x, Wq, bq, Wk, bk, Wv, bv, Wo, bo):
    bf = ml_dtypes.bfloat16
    xT = np.ascontiguousarray(np.asarray(x, dtype=np.float32)[0].T).astype(bf)
    Wq = np.asarray(Wq, dtype=np.float32)
    Wk = np.asarray(Wk, dtype=np.float32)
    Wv = np.asarray(Wv, dtype=np.float32)
    Wo = np.asarray(Wo, dtype=np.float32)
    bq = np.asarray(bq, dtype=np.float32)
    bk = np.asarray(bk, dtype=np.float32)
    bv = np.asarray(bv, dtype=np.float32)
    has_bias = bool(np.any(bq) or np.any(bk) or np.any(bv))
    in_maps = []
    for c in range(NCORES):
        qsl = slice(HPC * HD * c, HPC * HD * (c + 1))
        kv = c // 2
        ksl = slice(HD * kv, HD * (kv + 1))
        wk_dup = np.concatenate([Wk[:, ksl], Wk[:, ksl]], axis=1)
        bk_dup = np.concatenate([bk[ksl], bk[ksl]])
        in_maps.append({
            "xT": xT,
            "wq": np.ascontiguousarray(Wq[:, qsl]).astype(bf),
            "wk2": np.ascontiguousarray(wk_dup).astype(bf),
            "wv": np.ascontiguousarray(Wv[:, ksl]).astype(bf),
            "wo0": np.ascontiguousarray(
                Wo[HPC * HD * c:HPC * HD * c + HD, :]).astype(bf),
            "wo1": np.ascontiguousarray(
                Wo[HPC * HD * c + HD:HPC * HD * (c + 1), :]).astype(bf),
            "bq": bq[qsl].reshape(1, -1).astype(bf),
            "bk": bk_dup.reshape(1, -1).astype(bf),
            "bv": bv[ksl].reshape(1, -1).astype(bf),
        })
    return in_maps, has_bias


def run(inputs, trace=False):
    in_maps, has_bias = _prep_inputs(**inputs)
    key = ("nc", has_bias)
    if key not in _CACHE:
        _CACHE[key] = _build(has_bias)
    nc = _CACHE[key]
    res = run_bass_kernel_spmd(nc, in_maps, list(range(NCORES)), trace=trace)
    bo = np.asarray(inputs["bo"], dtype=np.float32)
    acc = np.zeros((L, D), dtype=np.float32)
    for r in res.results:
        acc += np.asarray(r["out"], dtype=np.float32)
    out = (acc + bo).reshape(1, L, D)
    return out, res


def kernel(**inputs):
    out, _ = run(inputs, trace=False)
    return out
